# revision 38
# baseline (speedup 1.0000x reference)
"""Trainium2 Bass kernel for a top-2 gated MoE layer (8 experts, H=1024, F=4096).

Strategy (expert parallelism across the 8 NeuronCores):
  - Host computes routing (top-2 argsort of gate logits), the exact top-2
    softmax gate weights (comb = score * alpha), and the LayerNorm of x
    (ln_w/ln_b folded into fc1 weights/bias).  Each expert's normalized
    tokens are gathered into a padded transposed block hdnT [H, C].
  - Each core runs one expert as a pure GEMM pipeline:
      fc1 (bf16, weight-stationary) -> gelu -> fc2 (PSUM-accumulated over
      all of F; last NT8 f-tiles as fp8e4 DoubleRow passes at 2x rate)
      -> (psum + b2) * comb -> bf16 output.
    No LN, gate, or partition ops on device; Scalar does gelu only, Vector
    does the finalize ops only.
  - Host scatter-adds the per-expert outputs back into the full [B,S,H].

Self-contained: shapes are hardcoded from the problem spec.
"""

import numpy as np
import ml_dtypes
from contextlib import ExitStack

TOP_K = 2
LN_EPS = 1e-5
B, S, H, E, F = 2, 2048, 1024, 8, 4096
T = B * S
P = 128
KH = H // P          # 8 H-tiles
FB = 1024            # fc1 F block size
NFB = F // FB        # 4 blocks
MF = FB // P         # 8 F-tiles per block
KF = F // P          # 32 F-tiles
NT8 = 6              # trailing f-tiles of fc2 in fp8 (3 DoubleRow passes)
KB16 = KF - NT8      # leading f-tiles kept bf16

_BUILD_CACHE = {}


def _chunks(C):
    # near-equal chunks, each <= 512 to fit one PSUM bank as fp32 and long
    # enough (> ~290 cols) that the next weight load hides under the last
    # matmul of the previous weight group
    n = max(3, -(-C // 512))
    base = C // n
    rem = C - base * n
    out = []
    off = 0
    for i in range(n):
        w = base + (1 if i < rem else 0)
        out.append((off, w))
        off += w
    return out


def _build(C):
    """Build + compile the single-core Bass program (SPMD across 8 cores)."""
    if C in _BUILD_CACHE:
        return _BUILD_CACHE[C]

    import concourse.bass as bass  # noqa: F401
    import concourse.tile as tile
    import concourse.mybir as mybir
    from concourse import bacc

    bf = mybir.dt.bfloat16
    f32 = mybir.dt.float32
    f8 = mybir.dt.float8e4
    AF = mybir.ActivationFunctionType
    OP = mybir.AluOpType
    DR = mybir.MatmulPerfMode.DoubleRow

    nc = bacc.Bacc("TRN2", target_bir_lowering=False, debug=False, num_devices=8)

    d_x = nc.dram_tensor("hdnT", [H, C], bf, kind="ExternalInput")
    d_w1 = nc.dram_tensor("w1", [H, F], bf, kind="ExternalInput")
    d_w2 = nc.dram_tensor("w2r", [P, KH * KB16, P], bf, kind="ExternalInput")
    d_w28 = nc.dram_tensor("w28r", [P, KH * NT8, P], f8, kind="ExternalInput")
    d_par = nc.dram_tensor("par", [P, C + 40], f32, kind="ExternalInput")
    d_y = nc.dram_tensor("ytT", [H, C], bf, kind="ExternalOutput")

    chunks = _chunks(C)
    WMAX = max(w for _, w in chunks)

    with tile.TileContext(nc) as tc, ExitStack() as ctx:
        const = ctx.enter_context(tc.tile_pool(name="const", bufs=1))
        hpool = ctx.enter_context(tc.tile_pool(name="hdn", bufs=1))
        w1pool = ctx.enter_context(tc.tile_pool(name="w1", bufs=2))
        w2pool = ctx.enter_context(tc.tile_pool(name="w2", bufs=5))
        apool = ctx.enter_context(tc.tile_pool(name="acts", bufs=1))
        ypool = ctx.enter_context(tc.tile_pool(name="y", bufs=1))
        psp = ctx.enter_context(tc.tile_pool(name="ps", bufs=8, space="PSUM"))

        # PE warm-up: junk matmuls train the clock up while DMAs fly.
        ones_k = const.tile([P, 1], bf)
        nc.vector.memset(ones_k, 1.0)
        warm_rhs = const.tile([P, WMAX], bf)
        nc.vector.memset(warm_rhs, 0.0)
        psw = psp.tile([P, WMAX], f32, tag="ps", name="warm")
        for _ in range(20):
            nc.tensor.matmul(psw[0:1, :], ones_k[:], warm_rhs[:],
                             start=True, stop=True)

        # ---- DMA issues, priority-ordered ----
        hdn = hpool.tile([P, KH, C], bf, tag="hdn", name="hdn")
        d_xr = d_x.ap().rearrange("(k p) c -> p k c", p=P)
        d_w1r = d_w1.ap().rearrange("(k p) f -> p k f", p=P)

        def load_w1_half(t, fb, half):
            fl = fb * FB + half * (FB // 2)
            nc.sync.dma_start(
                t[:, :, half * (FB // 2):(half + 1) * (FB // 2)],
                d_w1r[:, :, fl:fl + FB // 2])

        def load_w2(h):
            t = w2pool.tile([P, KB16, P], bf, tag="w2", name=f"w2_{h}")
            nc.sync.dma_start(t[:], d_w2.ap()[:, h * KB16:(h + 1) * KB16, :])
            return t

        def load_w1_piece(t, fb, fl, fh):
            nc.sync.dma_start(t[:, :, fl:fh],
                              d_w1r[:, :, fb * FB + fl:fb * FB + fh])

        w1t = [w1pool.tile([P, KH, FB], bf, tag="w1", name=f"w1_{fb}")
               if fb < 2 else None for fb in range(NFB)]
        # stream pieces in exact PE-consumption order
        (o0, wc0) = chunks[0]
        nc.sync.dma_start(hdn[:, :, 0:wc0], d_xr[:, :, 0:wc0])
        load_w1_piece(w1t[0], 0, 0, 256)          # m0,m1 of block 0
        par = const.tile([P, C + 40], f32)
        nc.sync.dma_start(par[:], d_par.ap())     # gates the first gelu
        load_w1_piece(w1t[0], 0, 256, 512)        # m2,m3
        for (off, w) in chunks[1:]:
            nc.sync.dma_start(hdn[:, :, off:off + w], d_xr[:, :, off:off + w])
        load_w1_piece(w1t[0], 0, 512, 1024)       # m4-7
        load_w1_half(w1t[1], 1, 0)
        load_w1_half(w1t[1], 1, 1)
        w28t = const.tile([P, KH * NT8, P], f8)
        nc.sync.dma_start(w28t[:], d_w28.ap())
        w2t = [None] * KH
        for h in range(5):
            w2t[h] = load_w2(h)

        a_big = apool.tile([P, KB16, C], bf, tag="a", name="a_big")
        a8 = apool.tile([P, NT8, C], f8, tag="a8", name="a8")
        y_big = ypool.tile([P, KH, C], bf, tag="y", name="y_big")
        d_yr = d_y.ap().rearrange("(k p) c -> p k c", p=P)

        # ---- fc1: weight-stationary — each lhsT feeds all chunks, so the
        # PE's weight loads amortize over the full C columns.  fb0 is split
        # finer (first m-half on chunk 0, then the rest) so compute starts
        # as soon as the first half-block of w1 and chunk 0 of hdn land. ----
        def fc1_group(fb, m, cis):
            fcol = fb * MF + m
            psg = {ci: psp.tile([P, WMAX], f32, tag="ps",
                                name=f"psa_{fcol}_{ci}")
                   for ci in cis}
            for k in range(KH):
                lhsT = w1t[fb][:, k, m * P:(m + 1) * P]
                for ci in cis:
                    off, w = chunks[ci]
                    nc.tensor.matmul(psg[ci][:, 0:w], lhsT,
                                     hdn[:, k, off:off + w],
                                     start=(k == 0), stop=(k == KH - 1))
            for ci in cis:
                off, w = chunks[ci]
                dst = (a_big[:, fcol, off:off + w] if fcol < KB16
                       else a8[:, fcol - KB16, off:off + w])
                nc.scalar.activation(dst, psg[ci][:, 0:w],
                                     AF.Gelu_apprx_tanh,
                                     bias=par[:, C + fcol:C + fcol + 1])

        ncis = list(range(len(chunks)))
        for m in range(MF // 2):
            fc1_group(0, m, [0])
        for m in range(MF // 2):
            fc1_group(0, m, ncis[1:])
        for m in range(MF // 2, MF):
            fc1_group(0, m, ncis)
        # fb0 done; issue w1 block 2/3 loads here so their ring-slot waits
        # don't sit in front of anything urgent on the Sync queue
        for fb in (2, 3):
            w1t[fb] = w1pool.tile([P, KH, FB], bf, tag="w1", name=f"w1_{fb}")
            load_w1_half(w1t[fb], fb, 0)
            load_w1_half(w1t[fb], fb, 1)
        for fb in range(1, NFB):
            for m in range(MF):
                fc1_group(fb, m, ncis)

        # ---- fc2: per output h-tile, full-F PSUM accumulation, finalize ----
        for h in range(KH):
            if h + 5 < KH:
                w2t[h + 5] = load_w2(h + 5)
            psg = [psp.tile([P, WMAX], f32, tag="ps", name=f"psy_{h}_{ci}")
                   for ci in range(len(chunks))]
            for kk in range(KB16):
                lhsT = w2t[h][:, kk, :]
                for ci, (off, w) in enumerate(chunks):
                    nc.tensor.matmul(psg[ci][:, 0:w], lhsT,
                                     a_big[:, kk, off:off + w],
                                     start=(kk == 0), stop=False)
            # trailing f-tiles: fp8 DoubleRow, two k-tiles per pass
            for j in range(NT8 // 2):
                lhsT = w28t[:, h * NT8 + 2 * j:h * NT8 + 2 * j + 2, :]
                for ci, (off, w) in enumerate(chunks):
                    nc.tensor.matmul(psg[ci][:, 0:w], lhsT,
                                     a8[:, 2 * j:2 * j + 2, off:off + w],
                                     start=False, stop=(j == NT8 // 2 - 1),
                                     perf_mode=DR)
            for ci, (off, w) in enumerate(chunks):
                # y = (psum + b2_h) * comb
                nc.vector.scalar_tensor_tensor(
                    y_big[:, h, off:off + w], psg[ci][:, 0:w],
                    par[:, C + 32 + h:C + 33 + h], par[:, off:off + w],
                    OP.add, OP.mult)
                if h == KH - 1:
                    nc.sync.dma_start(d_yr[:, h:h + 1, off:off + w],
                                      y_big[:, h:h + 1, off:off + w])
            if h < KH - 1:
                nc.sync.dma_start(d_yr[:, h:h + 1, :], y_big[:, h:h + 1, :])

    nc.compile()
    _BUILD_CACHE[C] = nc
    return nc


def _prepare(x, Wg, alpha, ln_w, ln_b, fc1_w, fc1_b, fc2_w, fc2_b):
    """Host-side routing, LN, gate weights + per-core input construction."""
    bfnp = ml_dtypes.bfloat16
    xf = np.asarray(x, np.float32).reshape(T, H)
    Wg = np.asarray(Wg, np.float32)
    alpha = np.asarray(alpha, np.float32)
    ln_w = np.asarray(ln_w, np.float32)
    ln_b = np.asarray(ln_b, np.float32)
    fc1_w = np.asarray(fc1_w, np.float32)
    fc1_b = np.asarray(fc1_b, np.float32)
    fc2_w = np.asarray(fc2_w, np.float32)
    fc2_b = np.asarray(fc2_b, np.float32)

    # routing (matches jax.lax.top_k tie-breaking) + exact top-2 softmax
    logits = xf @ Wg
    order = np.argsort(-logits, axis=1, kind="stable")
    top2 = order[:, :TOP_K]
    tv = np.take_along_axis(logits, top2, axis=1)
    sm = np.exp(tv - tv.max(1, keepdims=True))
    sm /= sm.sum(1, keepdims=True)
    comb = np.zeros((T, E), np.float32)
    np.put_along_axis(comb, top2, sm.astype(np.float32), axis=1)
    comb *= alpha
    sel = np.zeros((T, E), dtype=bool)
    sel[np.arange(T)[:, None], top2] = True
    idx = [np.nonzero(sel[:, e])[0] for e in range(E)]

    maxc = max(len(i) for i in idx)
    C = max(512, 4 * ((maxc + 3) // 4))

    # LayerNorm of x (expert-independent part)
    mu = xf.mean(1, keepdims=True)
    var = ((xf - mu) ** 2).mean(1, keepdims=True)
    hdn_base = (xf - mu) / np.sqrt(var + LN_EPS)

    in_maps = []
    for e in range(E):
        n = len(idx[e])
        # fold ln_w into fc1 weights, ln_b into fc1 bias
        if np.all(ln_w[e] == 1.0):
            w1 = fc1_w[e]
        else:
            w1 = ln_w[e][:, None] * fc1_w[e]
        b1 = fc1_b[e].astype(np.float32)
        if np.any(ln_b[e]):
            b1 = b1 + ln_b[e] @ w1

        hg = np.zeros((C, H), np.float32)
        hg[:n] = hdn_base[idx[e]]

        w2r = np.ascontiguousarray(
            fc2_w[e][:KB16 * P].reshape(KB16, P, KH, P).transpose(
                1, 2, 0, 3)).astype(bfnp)
        w28r = np.ascontiguousarray(
            fc2_w[e][KB16 * P:].reshape(NT8, P, KH, P).transpose(
                1, 2, 0, 3)).astype(ml_dtypes.float8_e4m3)

        par = np.zeros((P, C + 40), np.float32)
        par[:, :C][:, :n] = comb[idx[e], e]          # broadcast comb row
        par[:, C:C + 32] = b1.reshape(KF, P).T
        par[:, C + 32:C + 40] = fc2_b[e].reshape(KH, P).T

        in_maps.append({
            "hdnT": np.ascontiguousarray(hg.T).astype(bfnp),
            "w1": w1.astype(bfnp),
            "w2r": w2r,
            "w28r": w28r.reshape(P, KH * NT8, P),
            "par": np.ascontiguousarray(par),
        })
    return in_maps, idx, C


def _kernel_impl(inputs, trace=False, trace_cores=None):
    from concourse import bass_utils

    in_maps, idx, C = _prepare(**inputs)
    nc = _build(C)
    res = bass_utils.run_bass_kernel_spmd(
        nc, in_maps, core_ids=list(range(E)),
        trace=trace, trace_cores=trace_cores)

    out = np.zeros((T, H), np.float32)
    for e in range(E):
        yt = np.asarray(res.results[e]["ytT"], np.float32)  # [H, C]
        n = len(idx[e])
        out[idx[e]] += yt.T[:n]
    return out.reshape(B, S, H), res


def kernel(**inputs):
    out, _ = _kernel_impl(inputs)
    return out


# revision 42
# speedup vs baseline: 1.0238x; 1.0238x over previous
"""Trainium2 Bass kernel for a top-2 gated MoE layer (8 experts, H=1024, F=4096).

Strategy (expert parallelism across the 8 NeuronCores):
  - Host computes routing (top-2 argsort of gate logits), the exact top-2
    softmax gate weights (comb = score * alpha), and the LayerNorm of x
    (ln_w/ln_b folded into fc1 weights/bias).  Each expert's normalized
    tokens are gathered into a padded transposed block hdnT [H, C].
  - Each core runs one expert as a pure GEMM pipeline:
      fc1 (bf16, weight-stationary) -> gelu -> fc2 (PSUM-accumulated over
      all of F; last NT8 f-tiles as fp8e4 DoubleRow passes at 2x rate)
      -> (psum + b2) * comb -> bf16 output.
    No LN, gate, or partition ops on device; Scalar does gelu only, Vector
    does the finalize ops only.
  - Host scatter-adds the per-expert outputs back into the full [B,S,H].

Self-contained: shapes are hardcoded from the problem spec.
"""

import numpy as np
import ml_dtypes
from contextlib import ExitStack

TOP_K = 2
LN_EPS = 1e-5
B, S, H, E, F = 2, 2048, 1024, 8, 4096
T = B * S
P = 128
KH = H // P          # 8 H-tiles
FB = 1024            # fc1 F block size
NFB = F // FB        # 4 blocks
MF = FB // P         # 8 F-tiles per block
KF = F // P          # 32 F-tiles
NT8 = 8              # trailing f-tiles of fc2 in fp8 (4 DoubleRow passes)
KB16 = KF - NT8      # leading f-tiles kept bf16
SW8 = 4.0            # fp8 balance scale: a8 = gelu/SW8, w28 = w2*SW8

_BUILD_CACHE = {}


def _chunks(C):
    # near-equal chunks, each <= 512 to fit one PSUM bank as fp32 and long
    # enough (> ~290 cols) that the next weight load hides under the last
    # matmul of the previous weight group
    n = max(3, -(-C // 512))
    base = C // n
    rem = C - base * n
    out = []
    off = 0
    for i in range(n):
        w = base + (1 if i < rem else 0)
        out.append((off, w))
        off += w
    return out


def _build(C):
    """Build + compile the single-core Bass program (SPMD across 8 cores)."""
    if C in _BUILD_CACHE:
        return _BUILD_CACHE[C]

    import concourse.bass as bass  # noqa: F401
    import concourse.tile as tile
    import concourse.mybir as mybir
    from concourse import bacc

    bf = mybir.dt.bfloat16
    f32 = mybir.dt.float32
    f8 = mybir.dt.float8e4
    AF = mybir.ActivationFunctionType
    OP = mybir.AluOpType
    DR = mybir.MatmulPerfMode.DoubleRow

    nc = bacc.Bacc("TRN2", target_bir_lowering=False, debug=False, num_devices=8)

    d_x = nc.dram_tensor("hdnT", [H, C], bf, kind="ExternalInput")
    d_w1 = nc.dram_tensor("w1", [H, F], bf, kind="ExternalInput")
    d_w2 = nc.dram_tensor("w2r", [P, KH * KB16, P], bf, kind="ExternalInput")
    d_w28 = nc.dram_tensor("w28r", [P, KH * NT8, P], f8, kind="ExternalInput")
    d_par = nc.dram_tensor("par", [P, C + 40], f32, kind="ExternalInput")
    d_y = nc.dram_tensor("ytT", [H, C], bf, kind="ExternalOutput")

    chunks = _chunks(C)
    WMAX = max(w for _, w in chunks)

    with tile.TileContext(nc) as tc, ExitStack() as ctx:
        const = ctx.enter_context(tc.tile_pool(name="const", bufs=1))
        hpool = ctx.enter_context(tc.tile_pool(name="hdn", bufs=1))
        w1pool = ctx.enter_context(tc.tile_pool(name="w1", bufs=2))
        w2pool = ctx.enter_context(tc.tile_pool(name="w2", bufs=5))
        apool = ctx.enter_context(tc.tile_pool(name="acts", bufs=1))
        ypool = ctx.enter_context(tc.tile_pool(name="y", bufs=1))
        psp = ctx.enter_context(tc.tile_pool(name="ps", bufs=8, space="PSUM"))

        # PE warm-up: junk matmuls train the clock up while DMAs fly.
        ones_k = const.tile([P, 1], bf)
        nc.vector.memset(ones_k, 1.0)
        warm_rhs = const.tile([P, WMAX], bf)
        nc.vector.memset(warm_rhs, 0.0)
        psw = psp.tile([P, WMAX], f32, tag="ps", name="warm")
        for _ in range(20):
            nc.tensor.matmul(psw[0:1, :], ones_k[:], warm_rhs[:],
                             start=True, stop=True)

        # ---- DMA issues, priority-ordered ----
        hdn = hpool.tile([P, KH, C], bf, tag="hdn", name="hdn")
        d_xr = d_x.ap().rearrange("(k p) c -> p k c", p=P)
        d_w1r = d_w1.ap().rearrange("(k p) f -> p k f", p=P)

        def load_w1_half(t, fb, half):
            fl = fb * FB + half * (FB // 2)
            nc.sync.dma_start(
                t[:, :, half * (FB // 2):(half + 1) * (FB // 2)],
                d_w1r[:, :, fl:fl + FB // 2])

        def load_w2(h):
            t = w2pool.tile([P, KB16, P], bf, tag="w2", name=f"w2_{h}")
            nc.sync.dma_start(t[:], d_w2.ap()[:, h * KB16:(h + 1) * KB16, :])
            return t

        def load_w1_piece(t, fb, fl, fh):
            nc.sync.dma_start(t[:, :, fl:fh],
                              d_w1r[:, :, fb * FB + fl:fb * FB + fh])

        w1t = [w1pool.tile([P, KH, FB], bf, tag="w1", name=f"w1_{fb}")
               if fb < 2 else None for fb in range(NFB)]
        # stream pieces in exact PE-consumption order
        (o0, wc0) = chunks[0]
        nc.sync.dma_start(hdn[:, :, 0:wc0], d_xr[:, :, 0:wc0])
        load_w1_piece(w1t[0], 0, 0, 256)          # m0,m1 of block 0
        par = const.tile([P, C + 40], f32)
        nc.sync.dma_start(par[:], d_par.ap())     # gates the first gelu
        load_w1_piece(w1t[0], 0, 256, 512)        # m2,m3
        for (off, w) in chunks[1:]:
            nc.sync.dma_start(hdn[:, :, off:off + w], d_xr[:, :, off:off + w])
        load_w1_piece(w1t[0], 0, 512, 1024)       # m4-7
        load_w1_half(w1t[1], 1, 0)
        load_w1_half(w1t[1], 1, 1)
        w28t = const.tile([P, KH * NT8, P], f8)
        nc.sync.dma_start(w28t[:], d_w28.ap())
        w2t = [None] * KH
        for h in range(5):
            w2t[h] = load_w2(h)

        a_big = apool.tile([P, KF, C], bf, tag="a", name="a_big")
        a8 = apool.tile([P, NT8, C], f8, tag="a8", name="a8")
        y_big = ypool.tile([P, KH, C], bf, tag="y", name="y_big")
        d_yr = d_y.ap().rearrange("(k p) c -> p k c", p=P)

        # ---- fc1: weight-stationary — each lhsT feeds all chunks, so the
        # PE's weight loads amortize over the full C columns.  fb0 is split
        # finer (first m-half on chunk 0, then the rest) so compute starts
        # as soon as the first half-block of w1 and chunk 0 of hdn land. ----
        def fc1_group(fb, m, cis):
            fcol = fb * MF + m
            psg = {ci: psp.tile([P, WMAX], f32, tag="ps",
                                name=f"psa_{fcol}_{ci}")
                   for ci in cis}
            for k in range(KH):
                lhsT = w1t[fb][:, k, m * P:(m + 1) * P]
                for ci in cis:
                    off, w = chunks[ci]
                    nc.tensor.matmul(psg[ci][:, 0:w], lhsT,
                                     hdn[:, k, off:off + w],
                                     start=(k == 0), stop=(k == KH - 1))
            for ci in cis:
                off, w = chunks[ci]
                nc.scalar.activation(a_big[:, fcol, off:off + w],
                                     psg[ci][:, 0:w], AF.Gelu_apprx_tanh,
                                     bias=par[:, C + fcol:C + fcol + 1])
                if fcol >= KB16:
                    # balanced fp8: a8 = gelu/SW8 (host ships w2 tail * SW8)
                    nc.scalar.mul(a8[:, fcol - KB16, off:off + w],
                                  a_big[:, fcol, off:off + w], 1.0 / SW8)

        ncis = list(range(len(chunks)))
        for m in range(MF // 2):
            fc1_group(0, m, [0])
        for m in range(MF // 2):
            fc1_group(0, m, ncis[1:])
        for m in range(MF // 2, MF):
            fc1_group(0, m, ncis)
        # fb0 done; issue w1 block 2/3 loads here so their ring-slot waits
        # don't sit in front of anything urgent on the Sync queue
        for fb in (2, 3):
            w1t[fb] = w1pool.tile([P, KH, FB], bf, tag="w1", name=f"w1_{fb}")
            load_w1_half(w1t[fb], fb, 0)
            load_w1_half(w1t[fb], fb, 1)
        for fb in range(1, NFB):
            for m in range(MF):
                fc1_group(fb, m, ncis)

        # ---- fc2: per output h-tile, full-F PSUM accumulation, finalize ----
        for h in range(KH):
            if h + 5 < KH:
                w2t[h + 5] = load_w2(h + 5)
            psg = [psp.tile([P, WMAX], f32, tag="ps", name=f"psy_{h}_{ci}")
                   for ci in range(len(chunks))]
            for kk in range(KB16):
                lhsT = w2t[h][:, kk, :]
                for ci, (off, w) in enumerate(chunks):
                    nc.tensor.matmul(psg[ci][:, 0:w], lhsT,
                                     a_big[:, kk, off:off + w],
                                     start=(kk == 0), stop=False)
            # trailing f-tiles: fp8 DoubleRow, two k-tiles per pass
            for j in range(NT8 // 2):
                lhsT = w28t[:, h * NT8 + 2 * j:h * NT8 + 2 * j + 2, :]
                for ci, (off, w) in enumerate(chunks):
                    nc.tensor.matmul(psg[ci][:, 0:w], lhsT,
                                     a8[:, 2 * j:2 * j + 2, off:off + w],
                                     start=False, stop=(j == NT8 // 2 - 1),
                                     perf_mode=DR)
            for ci, (off, w) in enumerate(chunks):
                # y = (psum + b2_h) * comb
                nc.vector.scalar_tensor_tensor(
                    y_big[:, h, off:off + w], psg[ci][:, 0:w],
                    par[:, C + 32 + h:C + 33 + h], par[:, off:off + w],
                    OP.add, OP.mult)
                if h == KH - 1:
                    nc.sync.dma_start(d_yr[:, h:h + 1, off:off + w],
                                      y_big[:, h:h + 1, off:off + w])
            if h < KH - 1:
                nc.sync.dma_start(d_yr[:, h:h + 1, :], y_big[:, h:h + 1, :])

    nc.compile()
    _BUILD_CACHE[C] = nc
    return nc


def _prepare(x, Wg, alpha, ln_w, ln_b, fc1_w, fc1_b, fc2_w, fc2_b):
    """Host-side routing, LN, gate weights + per-core input construction."""
    bfnp = ml_dtypes.bfloat16
    xf = np.asarray(x, np.float32).reshape(T, H)
    Wg = np.asarray(Wg, np.float32)
    alpha = np.asarray(alpha, np.float32)
    ln_w = np.asarray(ln_w, np.float32)
    ln_b = np.asarray(ln_b, np.float32)
    fc1_w = np.asarray(fc1_w, np.float32)
    fc1_b = np.asarray(fc1_b, np.float32)
    fc2_w = np.asarray(fc2_w, np.float32)
    fc2_b = np.asarray(fc2_b, np.float32)

    # routing (matches jax.lax.top_k tie-breaking) + exact top-2 softmax
    logits = xf @ Wg
    order = np.argsort(-logits, axis=1, kind="stable")
    top2 = order[:, :TOP_K]
    tv = np.take_along_axis(logits, top2, axis=1)
    sm = np.exp(tv - tv.max(1, keepdims=True))
    sm /= sm.sum(1, keepdims=True)
    comb = np.zeros((T, E), np.float32)
    np.put_along_axis(comb, top2, sm.astype(np.float32), axis=1)
    comb *= alpha
    sel = np.zeros((T, E), dtype=bool)
    sel[np.arange(T)[:, None], top2] = True
    idx = [np.nonzero(sel[:, e])[0] for e in range(E)]

    maxc = max(len(i) for i in idx)
    C = max(512, 4 * ((maxc + 3) // 4))

    # LayerNorm of x (expert-independent part)
    mu = xf.mean(1, keepdims=True)
    var = ((xf - mu) ** 2).mean(1, keepdims=True)
    hdn_base = (xf - mu) / np.sqrt(var + LN_EPS)

    in_maps = []
    for e in range(E):
        n = len(idx[e])
        # fold ln_w into fc1 weights, ln_b into fc1 bias
        if np.all(ln_w[e] == 1.0):
            w1 = fc1_w[e]
        else:
            w1 = ln_w[e][:, None] * fc1_w[e]
        b1 = fc1_b[e].astype(np.float32)
        if np.any(ln_b[e]):
            b1 = b1 + ln_b[e] @ w1

        hg = np.zeros((C, H), np.float32)
        hg[:n] = hdn_base[idx[e]]

        w2r = np.ascontiguousarray(
            fc2_w[e][:KB16 * P].reshape(KB16, P, KH, P).transpose(
                1, 2, 0, 3)).astype(bfnp)
        w28r = np.ascontiguousarray(
            fc2_w[e][KB16 * P:].reshape(NT8, P, KH, P).transpose(
                1, 2, 0, 3) * SW8).astype(ml_dtypes.float8_e4m3)

        par = np.zeros((P, C + 40), np.float32)
        par[:, :C][:, :n] = comb[idx[e], e]          # broadcast comb row
        par[:, C:C + 32] = b1.reshape(KF, P).T
        par[:, C + 32:C + 40] = fc2_b[e].reshape(KH, P).T

        in_maps.append({
            "hdnT": np.ascontiguousarray(hg.T).astype(bfnp),
            "w1": w1.astype(bfnp),
            "w2r": w2r,
            "w28r": w28r.reshape(P, KH * NT8, P),
            "par": np.ascontiguousarray(par),
        })
    return in_maps, idx, C


def _kernel_impl(inputs, trace=False, trace_cores=None):
    from concourse import bass_utils

    in_maps, idx, C = _prepare(**inputs)
    nc = _build(C)
    res = bass_utils.run_bass_kernel_spmd(
        nc, in_maps, core_ids=list(range(E)),
        trace=trace, trace_cores=trace_cores)

    out = np.zeros((T, H), np.float32)
    for e in range(E):
        yt = np.asarray(res.results[e]["ytT"], np.float32)  # [H, C]
        n = len(idx[e])
        out[idx[e]] += yt.T[:n]
    return out.reshape(B, S, H), res


def kernel(**inputs):
    out, _ = _kernel_impl(inputs)
    return out
